# revision 25
# baseline (speedup 1.0000x reference)
"""Trainium2 Bass kernel for per-pixel kernel application (KPN-style ApplyKernel).

y[c,h,w] = sum_{ii,jj} xpad[c, h+ii, w+jj] * k[ii*11+jj, h, w]

Strategy (8 NeuronCores, data-parallel over H strips of 90 rows):
  - Partition p owns a 10-column block of W (128 partitions x 10 = 1280), with
    the +-5 column halo stored in the free dim, so both row and column shifts
    of a tap are plain access-pattern offsets (DVE lanes are partition-locked,
    so shifts must live in the free dim). All 128 lanes are used.
  - Host: pad x and build bf16 slabs [128, 3ch x 100rows x 20cols] in two
    column-alignment variants so every tap's VectorE read stays 4-byte
    aligned, keeping tensor_tensor in its 2x bf16 mode. k is re-laid-out
    host-side to [128, 121, 900] (partition-block-major, even-column taps
    first) so DMA descriptors are large contiguous chunks.
  - Device, per group of taps: one SWDGE DMA of the k group (f32->bf16 cast
    in flight); per tap one VectorE tensor_tensor multiply (bf16 2x mode) and
    6 TensorE identity-matmuls (K=M=128) accumulating into 6 PSUM banks
    (3 channels x 512/388-col chunks).
  - Epilogue: ScalarE+VectorE evacuate PSUM -> SBUF per channel, overlapped
    per-channel DMAs out, host-side reshape of y.
"""

import sys

if "/opt/trn_rl_repo" not in sys.path:
    sys.path.insert(0, "/opt/trn_rl_repo")

import numpy as np
import ml_dtypes

import concourse.mybir as mybir
from concourse import bacc
from concourse.tile import TileContext
from concourse.bass_utils import run_bass_kernel_spmd

KS = 11
HALF = 5
H, W, C = 720, 1280, 3
NCORES = 8
HS = H // NCORES            # 90 rows per core
NP = 128                    # partitions (one 10-col block each)
CPP = W // NP               # 10 output cols per partition
ROWS_ST = HS + 2 * HALF     # 100 rows stored per partition
COLS_ST = CPP + 2 * HALF    # 20 cols stored per partition
SLABF = C * ROWS_ST * COLS_ST   # 6000 bf16 per partition per variant
NTAPS = KS * KS             # 121
FD = HS * CPP               # 900 elements per channel per tap
PFD = C * FD                # 2700 product elements per tap
N0, N1 = 512, FD - 512      # matmul chunk widths per channel (512, 388)

# tap order: even-jj taps first (only need slab variant 0), then odd-jj
TAP_PERM = ([t for t in range(NTAPS) if (t % KS) % 2 == 0]
            + [t for t in range(NTAPS) if (t % KS) % 2 == 1])
# k DMA group sizes over the permuted order (66 even + 55 odd taps);
# ramped start so the pipeline fills before the big groups are needed
GROUPS = [2, 4] + [8] * 7 + [4] + [8] * 6 + [7]
assert sum(GROUPS) == NTAPS and sum(GROUPS[:10]) == 66

BF16 = ml_dtypes.bfloat16

_CACHE = {}


def _build_nc(taps=NTAPS):
    nc = bacc.Bacc("TRN2", target_bir_lowering=False, debug=False)
    k_d = nc.dram_tensor("k", [NP, NTAPS, FD], mybir.dt.float32, kind="ExternalInput")
    xs_d = nc.dram_tensor("xs", [2, NP, SLABF], mybir.dt.bfloat16, kind="ExternalInput")
    id_d = nc.dram_tensor("ident", [NP, NP], mybir.dt.bfloat16, kind="ExternalInput")
    y_d = nc.dram_tensor("y", [NP, PFD], mybir.dt.float32, kind="ExternalOutput")

    with TileContext(nc) as tc:
        with tc.tile_pool(name="const", bufs=1) as const_pool, \
             tc.tile_pool(name="kbf", bufs=3) as kb_pool, \
             tc.tile_pool(name="prod", bufs=6) as prod_pool, \
             tc.tile_pool(name="out", bufs=1) as out_pool, \
             tc.tile_pool(name="psum", bufs=1, space="PSUM") as psum_pool:

            slab0 = const_pool.tile([NP, SLABF], mybir.dt.bfloat16)
            slab1 = const_pool.tile([NP, SLABF], mybir.dt.bfloat16)
            ident = const_pool.tile([NP, NP], mybir.dt.bfloat16)
            nc.sync.dma_start(ident[:], id_d.ap())
            nc.sync.dma_start(slab0[:], xs_d.ap()[0])
            nc.sync.dma_start(slab1[:], xs_d.ap()[1])
            slab_views = [
                s[:].rearrange("p (c r w) -> p c r w", c=C, r=ROWS_ST, w=COLS_ST)
                for s in (slab0, slab1)]

            accs = []
            for c in range(C):
                a0 = psum_pool.tile([NP, N0], mybir.dt.float32, name=f"acc{c}0")
                a1 = psum_pool.tile([NP, N1], mybir.dt.float32, name=f"acc{c}1")
                accs.append((a0, a1))

            gi0 = 0
            groups = []
            for ng in GROUPS:
                if gi0 >= taps:
                    break
                groups.append((gi0, min(ng, taps - gi0)))
                gi0 += ng
            for gi0, ng in groups:
                kb = kb_pool.tile([NP, ng * FD], mybir.dt.bfloat16, name="kb")
                nc.gpsimd.dma_start(
                    kb[:].rearrange("p (t f) -> p t f", t=ng),
                    k_d.ap()[:, gi0:gi0 + ng, :])

                for dt_ in range(ng):
                    gi = gi0 + dt_
                    t = TAP_PERM[gi]
                    ii, jj = divmod(t, KS)
                    v = jj & 1
                    jj2 = jj - v
                    xs_op = slab_views[v][:, :, ii:ii + HS, jj2:jj2 + CPP]
                    k_op = (kb[:, dt_ * FD:(dt_ + 1) * FD]
                            .rearrange("p (r w) -> p r w", r=HS)
                            .unsqueeze(1).broadcast_to([NP, C, HS, CPP]))
                    prod = prod_pool.tile([NP, PFD], mybir.dt.bfloat16, name="prod")
                    prod_view = prod[:].rearrange(
                        "p (c r w) -> p c r w", c=C, r=HS, w=CPP)
                    nc.vector.tensor_tensor(prod_view, xs_op, k_op,
                                            mybir.AluOpType.mult)
                    first = (gi == 0)
                    last = (gi == taps - 1)
                    for c in range(C):
                        nc.tensor.matmul(accs[c][0][:], ident[:],
                                         prod[:, c * FD:c * FD + N0],
                                         start=first, stop=last)
                        nc.tensor.matmul(accs[c][1][:], ident[:],
                                         prod[:, c * FD + N0:(c + 1) * FD],
                                         start=first, stop=last)

            yst = out_pool.tile([NP, PFD], mybir.dt.float32)
            for c in range(C):
                nc.scalar.copy(yst[:, c * FD:c * FD + N0], accs[c][0][:])
                nc.vector.tensor_copy(yst[:, c * FD + N0:(c + 1) * FD],
                                      accs[c][1][:])
                nc.sync.dma_start(y_d.ap()[:, c * FD:(c + 1) * FD],
                                  yst[:, c * FD:(c + 1) * FD])

    nc.compile()
    return nc


def get_nc(taps=NTAPS):
    if taps not in _CACHE:
        _CACHE[taps] = _build_nc(taps)
    return _CACHE[taps]


def _prep_inputs(x, k, padding, padding_value):
    """Host-side prep: pad x, build bf16 slabs + per-core shards."""
    x = np.asarray(x, dtype=np.float32)
    k = np.asarray(k, dtype=np.float32)
    pad = bool(int(np.asarray(padding)))
    pv = float(np.asarray(padding_value))

    if pad:
        assert x.shape == (1, C, H, W), x.shape
        xp = np.full((C, H + 2 * HALF, W + 2 * HALF + 1), 0.0, dtype=np.float32)
        xp[:, :, :W + 2 * HALF] = pv
        xp[:, HALF:HALF + H, HALF:HALF + W] = x[0]
    else:
        assert x.shape == (1, C, H + 2 * HALF, W + 2 * HALF), x.shape
        xp = np.zeros((C, H + 2 * HALF, W + 2 * HALF + 1), dtype=np.float32)
        xp[:, :, :W + 2 * HALF] = x[0]

    assert k.shape == (1, NTAPS, H, W), k.shape
    # partition-block-major, tap-permuted k: [core, p, t, (r w)]
    kt_all = np.ascontiguousarray(
        k[0][TAP_PERM].reshape(NTAPS, NCORES, HS, NP, CPP)
        .transpose(1, 3, 0, 2, 4)).reshape(NCORES, NP, NTAPS, FD)

    cols_idx = CPP * np.arange(NP)[:, None] + np.arange(COLS_ST)[None, :]
    ident = np.eye(NP, dtype=BF16)
    in_maps = []
    for ci in range(NCORES):
        rows = slice(HS * ci, HS * ci + ROWS_ST)
        xs = np.empty((2, NP, SLABF), dtype=BF16)
        for v in (0, 1):
            sv = xp[:, rows, v:v + W + 2 * HALF]           # [C, 100, 1290]
            win = sv[:, :, cols_idx]                       # [C, 100, 128, 20]
            xs[v] = win.transpose(2, 0, 1, 3).reshape(NP, SLABF).astype(BF16)
        in_maps.append({"k": kt_all[ci], "xs": xs, "ident": ident})
    return in_maps


def _assemble_y(results):
    """results[ci]["y"] is [128, 2700]; reassemble to [1, C, H, W]."""
    y = np.empty((C, H, W), dtype=np.float32)
    for ci in range(NCORES):
        blk = results[ci]["y"].reshape(NP, C, HS, CPP)     # [p, c, r, w]
        y[:, HS * ci:HS * (ci + 1), :] = (
            blk.transpose(1, 2, 0, 3).reshape(C, HS, W))
    return y[None]


def kernel(x, k, padding, padding_value):
    in_maps = _prep_inputs(x, k, padding, padding_value)
    nc = get_nc()
    res = run_bass_kernel_spmd(nc, in_maps, core_ids=list(range(NCORES)))
    return _assemble_y(res.results).astype(np.float32)


# revision 26
# speedup vs baseline: 1.0019x; 1.0019x over previous
"""Trainium2 Bass kernel for per-pixel kernel application (KPN-style ApplyKernel).

y[c,h,w] = sum_{ii,jj} xpad[c, h+ii, w+jj] * k[ii*11+jj, h, w]

Strategy (8 NeuronCores, data-parallel over H strips of 90 rows):
  - Partition p owns a 10-column block of W (128 partitions x 10 = 1280), with
    the +-5 column halo stored in the free dim, so both row and column shifts
    of a tap are plain access-pattern offsets (DVE lanes are partition-locked,
    so shifts must live in the free dim). All 128 lanes are used.
  - Host: pad x and build bf16 slabs [128, 3ch x 100rows x 20cols] in two
    column-alignment variants so every tap's VectorE read stays 4-byte
    aligned, keeping tensor_tensor in its 2x bf16 mode. k is re-laid-out
    host-side to [128, 121, 900] (partition-block-major, even-column taps
    first) so DMA descriptors are large contiguous chunks.
  - Device, per group of taps: one SWDGE DMA of the k group (f32->bf16 cast
    in flight); per tap one VectorE tensor_tensor multiply (bf16 2x mode) and
    6 TensorE identity-matmuls (K=M=128) accumulating into 6 PSUM banks
    (3 channels x 512/388-col chunks).
  - Epilogue: ScalarE+VectorE evacuate PSUM -> SBUF per channel, overlapped
    per-channel DMAs out, host-side reshape of y.
"""

import sys

if "/opt/trn_rl_repo" not in sys.path:
    sys.path.insert(0, "/opt/trn_rl_repo")

import numpy as np
import ml_dtypes

import concourse.mybir as mybir
from concourse import bacc
from concourse.tile import TileContext
from concourse.bass_utils import run_bass_kernel_spmd

KS = 11
HALF = 5
H, W, C = 720, 1280, 3
NCORES = 8
HS = H // NCORES            # 90 rows per core
NP = 128                    # partitions (one 10-col block each)
CPP = W // NP               # 10 output cols per partition
ROWS_ST = HS + 2 * HALF     # 100 rows stored per partition
COLS_ST = CPP + 2 * HALF    # 20 cols stored per partition
SLABF = C * ROWS_ST * COLS_ST   # 6000 bf16 per partition per variant
NTAPS = KS * KS             # 121
FD = HS * CPP               # 900 elements per channel per tap
PFD = C * FD                # 2700 product elements per tap
N0, N1 = 512, FD - 512      # matmul chunk widths per channel (512, 388)

# tap order: even-jj taps first (only need slab variant 0), then odd-jj
TAP_PERM = ([t for t in range(NTAPS) if (t % KS) % 2 == 0]
            + [t for t in range(NTAPS) if (t % KS) % 2 == 1])
# k DMA group sizes over the permuted order (66 even + 55 odd taps);
# ramped start so the pipeline fills before the big groups are needed
GROUPS = [2, 2, 4, 4] + [8] * 6 + [6] + [8] * 6 + [7]
assert sum(GROUPS) == NTAPS and sum(GROUPS[:11]) == 66

BF16 = ml_dtypes.bfloat16

_CACHE = {}


def _build_nc(taps=NTAPS):
    nc = bacc.Bacc("TRN2", target_bir_lowering=False, debug=False)
    k_d = nc.dram_tensor("k", [NP, NTAPS, FD], mybir.dt.float32, kind="ExternalInput")
    xs_d = nc.dram_tensor("xs", [2, NP, SLABF], mybir.dt.bfloat16, kind="ExternalInput")
    id_d = nc.dram_tensor("ident", [NP, NP], mybir.dt.bfloat16, kind="ExternalInput")
    y_d = nc.dram_tensor("y", [NP, PFD], mybir.dt.float32, kind="ExternalOutput")

    with TileContext(nc) as tc:
        with tc.tile_pool(name="const", bufs=1) as const_pool, \
             tc.tile_pool(name="kbf", bufs=3) as kb_pool, \
             tc.tile_pool(name="prod", bufs=6) as prod_pool, \
             tc.tile_pool(name="out", bufs=1) as out_pool, \
             tc.tile_pool(name="psum", bufs=1, space="PSUM") as psum_pool:

            slab0 = const_pool.tile([NP, SLABF], mybir.dt.bfloat16)
            slab1 = const_pool.tile([NP, SLABF], mybir.dt.bfloat16)
            ident = const_pool.tile([NP, NP], mybir.dt.bfloat16)
            nc.sync.dma_start(ident[:], id_d.ap())
            nc.sync.dma_start(slab0[:], xs_d.ap()[0])
            nc.sync.dma_start(slab1[:], xs_d.ap()[1])
            slab_views = [
                s[:].rearrange("p (c r w) -> p c r w", c=C, r=ROWS_ST, w=COLS_ST)
                for s in (slab0, slab1)]

            accs = []
            for c in range(C):
                a0 = psum_pool.tile([NP, N0], mybir.dt.float32, name=f"acc{c}0")
                a1 = psum_pool.tile([NP, N1], mybir.dt.float32, name=f"acc{c}1")
                accs.append((a0, a1))

            gi0 = 0
            groups = []
            for ng in GROUPS:
                if gi0 >= taps:
                    break
                groups.append((gi0, min(ng, taps - gi0)))
                gi0 += ng
            for gi0, ng in groups:
                kb = kb_pool.tile([NP, ng * FD], mybir.dt.bfloat16, name="kb")
                nc.gpsimd.dma_start(
                    kb[:].rearrange("p (t f) -> p t f", t=ng),
                    k_d.ap()[:, gi0:gi0 + ng, :])

                for dt_ in range(ng):
                    gi = gi0 + dt_
                    t = TAP_PERM[gi]
                    ii, jj = divmod(t, KS)
                    v = jj & 1
                    jj2 = jj - v
                    xs_op = slab_views[v][:, :, ii:ii + HS, jj2:jj2 + CPP]
                    k_op = (kb[:, dt_ * FD:(dt_ + 1) * FD]
                            .rearrange("p (r w) -> p r w", r=HS)
                            .unsqueeze(1).broadcast_to([NP, C, HS, CPP]))
                    prod = prod_pool.tile([NP, PFD], mybir.dt.bfloat16, name="prod")
                    prod_view = prod[:].rearrange(
                        "p (c r w) -> p c r w", c=C, r=HS, w=CPP)
                    nc.vector.tensor_tensor(prod_view, xs_op, k_op,
                                            mybir.AluOpType.mult)
                    first = (gi == 0)
                    last = (gi == taps - 1)
                    for c in range(C):
                        nc.tensor.matmul(accs[c][0][:], ident[:],
                                         prod[:, c * FD:c * FD + N0],
                                         start=first, stop=last)
                        nc.tensor.matmul(accs[c][1][:], ident[:],
                                         prod[:, c * FD + N0:(c + 1) * FD],
                                         start=first, stop=last)

            yst = out_pool.tile([NP, PFD], mybir.dt.float32)
            for c in range(C):
                nc.scalar.copy(yst[:, c * FD:c * FD + N0], accs[c][0][:])
                nc.vector.tensor_copy(yst[:, c * FD + N0:(c + 1) * FD],
                                      accs[c][1][:])
                nc.sync.dma_start(y_d.ap()[:, c * FD:(c + 1) * FD],
                                  yst[:, c * FD:(c + 1) * FD])

    nc.compile()
    return nc


def get_nc(taps=NTAPS):
    if taps not in _CACHE:
        _CACHE[taps] = _build_nc(taps)
    return _CACHE[taps]


def _prep_inputs(x, k, padding, padding_value):
    """Host-side prep: pad x, build bf16 slabs + per-core shards."""
    x = np.asarray(x, dtype=np.float32)
    k = np.asarray(k, dtype=np.float32)
    pad = bool(int(np.asarray(padding)))
    pv = float(np.asarray(padding_value))

    if pad:
        assert x.shape == (1, C, H, W), x.shape
        xp = np.full((C, H + 2 * HALF, W + 2 * HALF + 1), 0.0, dtype=np.float32)
        xp[:, :, :W + 2 * HALF] = pv
        xp[:, HALF:HALF + H, HALF:HALF + W] = x[0]
    else:
        assert x.shape == (1, C, H + 2 * HALF, W + 2 * HALF), x.shape
        xp = np.zeros((C, H + 2 * HALF, W + 2 * HALF + 1), dtype=np.float32)
        xp[:, :, :W + 2 * HALF] = x[0]

    assert k.shape == (1, NTAPS, H, W), k.shape
    # partition-block-major, tap-permuted k: [core, p, t, (r w)]
    kt_all = np.ascontiguousarray(
        k[0][TAP_PERM].reshape(NTAPS, NCORES, HS, NP, CPP)
        .transpose(1, 3, 0, 2, 4)).reshape(NCORES, NP, NTAPS, FD)

    cols_idx = CPP * np.arange(NP)[:, None] + np.arange(COLS_ST)[None, :]
    ident = np.eye(NP, dtype=BF16)
    in_maps = []
    for ci in range(NCORES):
        rows = slice(HS * ci, HS * ci + ROWS_ST)
        xs = np.empty((2, NP, SLABF), dtype=BF16)
        for v in (0, 1):
            sv = xp[:, rows, v:v + W + 2 * HALF]           # [C, 100, 1290]
            win = sv[:, :, cols_idx]                       # [C, 100, 128, 20]
            xs[v] = win.transpose(2, 0, 1, 3).reshape(NP, SLABF).astype(BF16)
        in_maps.append({"k": kt_all[ci], "xs": xs, "ident": ident})
    return in_maps


def _assemble_y(results):
    """results[ci]["y"] is [128, 2700]; reassemble to [1, C, H, W]."""
    y = np.empty((C, H, W), dtype=np.float32)
    for ci in range(NCORES):
        blk = results[ci]["y"].reshape(NP, C, HS, CPP)     # [p, c, r, w]
        y[:, HS * ci:HS * (ci + 1), :] = (
            blk.transpose(1, 2, 0, 3).reshape(C, HS, W))
    return y[None]


def kernel(x, k, padding, padding_value):
    in_maps = _prep_inputs(x, k, padding, padding_value)
    nc = get_nc()
    res = run_bass_kernel_spmd(nc, in_maps, core_ids=list(range(NCORES)))
    return _assemble_y(res.results).astype(np.float32)


# revision 27
# speedup vs baseline: 1.0102x; 1.0083x over previous
"""Trainium2 Bass kernel for per-pixel kernel application (KPN-style ApplyKernel).

y[c,h,w] = sum_{ii,jj} xpad[c, h+ii, w+jj] * k[ii*11+jj, h, w]

Strategy (8 NeuronCores, data-parallel over H strips of 90 rows):
  - Partition p owns a 10-column block of W (128 partitions x 10 = 1280), with
    the +-5 column halo stored in the free dim, so both row and column shifts
    of a tap are plain access-pattern offsets (DVE lanes are partition-locked,
    so shifts must live in the free dim). All 128 lanes are used.
  - Host: pad x and build bf16 slabs [128, 3ch x 100rows x 20cols] in two
    column-alignment variants so every tap's VectorE read stays 4-byte
    aligned, keeping tensor_tensor in its 2x bf16 mode. k is re-laid-out
    host-side to [128, 121, 900] (partition-block-major, even-column taps
    first) so DMA descriptors are large contiguous chunks.
  - Device, per group of taps: one SWDGE DMA of the k group (f32->bf16 cast
    in flight); per tap one VectorE tensor_tensor multiply (bf16 2x mode) and
    6 TensorE identity-matmuls (K=M=128) accumulating into 6 PSUM banks
    (3 channels x 512/388-col chunks).
  - Epilogue: ScalarE+VectorE evacuate PSUM -> SBUF per channel, overlapped
    per-channel DMAs out, host-side reshape of y.
"""

import sys

if "/opt/trn_rl_repo" not in sys.path:
    sys.path.insert(0, "/opt/trn_rl_repo")

import numpy as np
import ml_dtypes

import concourse.mybir as mybir
from concourse import bacc
from concourse.tile import TileContext
from concourse.bass_utils import run_bass_kernel_spmd

KS = 11
HALF = 5
H, W, C = 720, 1280, 3
NCORES = 8
HS = H // NCORES            # 90 rows per core
NP = 128                    # partitions (one 10-col block each)
CPP = W // NP               # 10 output cols per partition
ROWS_ST = HS + 2 * HALF     # 100 rows stored per partition
COLS_ST = CPP + 2 * HALF    # 20 cols stored per partition
SLABF = C * ROWS_ST * COLS_ST   # 6000 bf16 per partition per variant
NTAPS = KS * KS             # 121
FD = HS * CPP               # 900 elements per channel per tap
PFD = C * FD                # 2700 product elements per tap
N0, N1 = 512, FD - 512      # matmul chunk widths per channel (512, 388)

# tap order: even-jj taps first (only need slab variant 0), then odd-jj
TAP_PERM = ([t for t in range(NTAPS) if (t % KS) % 2 == 0]
            + [t for t in range(NTAPS) if (t % KS) % 2 == 1])
# k DMA group sizes over the permuted order (66 even + 55 odd taps);
# ramped start so the pipeline fills before the big groups are needed
GROUPS = [2, 2, 2, 2, 4, 4] + [8] * 6 + [2] + [8] * 6 + [7]
assert sum(GROUPS) == NTAPS and sum(GROUPS[:13]) == 66

BF16 = ml_dtypes.bfloat16

_CACHE = {}


def _build_nc(taps=NTAPS):
    nc = bacc.Bacc("TRN2", target_bir_lowering=False, debug=False)
    k_d = nc.dram_tensor("k", [NP, NTAPS, FD], mybir.dt.float32, kind="ExternalInput")
    xs_d = nc.dram_tensor("xs", [2, NP, SLABF], mybir.dt.bfloat16, kind="ExternalInput")
    id_d = nc.dram_tensor("ident", [NP, NP], mybir.dt.bfloat16, kind="ExternalInput")
    y_d = nc.dram_tensor("y", [NP, PFD], mybir.dt.float32, kind="ExternalOutput")

    with TileContext(nc) as tc:
        with tc.tile_pool(name="const", bufs=1) as const_pool, \
             tc.tile_pool(name="kbf", bufs=3) as kb_pool, \
             tc.tile_pool(name="prod", bufs=6) as prod_pool, \
             tc.tile_pool(name="out", bufs=1) as out_pool, \
             tc.tile_pool(name="psum", bufs=1, space="PSUM") as psum_pool:

            slab0 = const_pool.tile([NP, SLABF], mybir.dt.bfloat16)
            slab1 = const_pool.tile([NP, SLABF], mybir.dt.bfloat16)
            ident = const_pool.tile([NP, NP], mybir.dt.bfloat16)
            nc.sync.dma_start(ident[:], id_d.ap())
            nc.sync.dma_start(slab0[:], xs_d.ap()[0])
            nc.sync.dma_start(slab1[:], xs_d.ap()[1])
            slab_views = [
                s[:].rearrange("p (c r w) -> p c r w", c=C, r=ROWS_ST, w=COLS_ST)
                for s in (slab0, slab1)]

            accs = []
            for c in range(C):
                a0 = psum_pool.tile([NP, N0], mybir.dt.float32, name=f"acc{c}0")
                a1 = psum_pool.tile([NP, N1], mybir.dt.float32, name=f"acc{c}1")
                accs.append((a0, a1))

            gi0 = 0
            groups = []
            for ng in GROUPS:
                if gi0 >= taps:
                    break
                groups.append((gi0, min(ng, taps - gi0)))
                gi0 += ng
            for gi0, ng in groups:
                kb = kb_pool.tile([NP, ng * FD], mybir.dt.bfloat16, name="kb")
                nc.gpsimd.dma_start(
                    kb[:].rearrange("p (t f) -> p t f", t=ng),
                    k_d.ap()[:, gi0:gi0 + ng, :])

                for dt_ in range(ng):
                    gi = gi0 + dt_
                    t = TAP_PERM[gi]
                    ii, jj = divmod(t, KS)
                    v = jj & 1
                    jj2 = jj - v
                    xs_op = slab_views[v][:, :, ii:ii + HS, jj2:jj2 + CPP]
                    k_op = (kb[:, dt_ * FD:(dt_ + 1) * FD]
                            .rearrange("p (r w) -> p r w", r=HS)
                            .unsqueeze(1).broadcast_to([NP, C, HS, CPP]))
                    prod = prod_pool.tile([NP, PFD], mybir.dt.bfloat16, name="prod")
                    prod_view = prod[:].rearrange(
                        "p (c r w) -> p c r w", c=C, r=HS, w=CPP)
                    nc.vector.tensor_tensor(prod_view, xs_op, k_op,
                                            mybir.AluOpType.mult)
                    first = (gi == 0)
                    last = (gi == taps - 1)
                    for c in range(C):
                        nc.tensor.matmul(accs[c][0][:], ident[:],
                                         prod[:, c * FD:c * FD + N0],
                                         start=first, stop=last)
                        nc.tensor.matmul(accs[c][1][:], ident[:],
                                         prod[:, c * FD + N0:(c + 1) * FD],
                                         start=first, stop=last)

            yst = out_pool.tile([NP, PFD], mybir.dt.float32)
            for c in range(C):
                nc.scalar.copy(yst[:, c * FD:c * FD + N0], accs[c][0][:])
                nc.vector.tensor_copy(yst[:, c * FD + N0:(c + 1) * FD],
                                      accs[c][1][:])
                nc.sync.dma_start(y_d.ap()[:, c * FD:(c + 1) * FD],
                                  yst[:, c * FD:(c + 1) * FD])

    nc.compile()
    return nc


def get_nc(taps=NTAPS):
    if taps not in _CACHE:
        _CACHE[taps] = _build_nc(taps)
    return _CACHE[taps]


def _prep_inputs(x, k, padding, padding_value):
    """Host-side prep: pad x, build bf16 slabs + per-core shards."""
    x = np.asarray(x, dtype=np.float32)
    k = np.asarray(k, dtype=np.float32)
    pad = bool(int(np.asarray(padding)))
    pv = float(np.asarray(padding_value))

    if pad:
        assert x.shape == (1, C, H, W), x.shape
        xp = np.full((C, H + 2 * HALF, W + 2 * HALF + 1), 0.0, dtype=np.float32)
        xp[:, :, :W + 2 * HALF] = pv
        xp[:, HALF:HALF + H, HALF:HALF + W] = x[0]
    else:
        assert x.shape == (1, C, H + 2 * HALF, W + 2 * HALF), x.shape
        xp = np.zeros((C, H + 2 * HALF, W + 2 * HALF + 1), dtype=np.float32)
        xp[:, :, :W + 2 * HALF] = x[0]

    assert k.shape == (1, NTAPS, H, W), k.shape
    # partition-block-major, tap-permuted k: [core, p, t, (r w)]
    kt_all = np.ascontiguousarray(
        k[0][TAP_PERM].reshape(NTAPS, NCORES, HS, NP, CPP)
        .transpose(1, 3, 0, 2, 4)).reshape(NCORES, NP, NTAPS, FD)

    cols_idx = CPP * np.arange(NP)[:, None] + np.arange(COLS_ST)[None, :]
    ident = np.eye(NP, dtype=BF16)
    in_maps = []
    for ci in range(NCORES):
        rows = slice(HS * ci, HS * ci + ROWS_ST)
        xs = np.empty((2, NP, SLABF), dtype=BF16)
        for v in (0, 1):
            sv = xp[:, rows, v:v + W + 2 * HALF]           # [C, 100, 1290]
            win = sv[:, :, cols_idx]                       # [C, 100, 128, 20]
            xs[v] = win.transpose(2, 0, 1, 3).reshape(NP, SLABF).astype(BF16)
        in_maps.append({"k": kt_all[ci], "xs": xs, "ident": ident})
    return in_maps


def _assemble_y(results):
    """results[ci]["y"] is [128, 2700]; reassemble to [1, C, H, W]."""
    y = np.empty((C, H, W), dtype=np.float32)
    for ci in range(NCORES):
        blk = results[ci]["y"].reshape(NP, C, HS, CPP)     # [p, c, r, w]
        y[:, HS * ci:HS * (ci + 1), :] = (
            blk.transpose(1, 2, 0, 3).reshape(C, HS, W))
    return y[None]


def kernel(x, k, padding, padding_value):
    in_maps = _prep_inputs(x, k, padding, padding_value)
    nc = get_nc()
    res = run_bass_kernel_spmd(nc, in_maps, core_ids=list(range(NCORES)))
    return _assemble_y(res.results).astype(np.float32)


# revision 29
# speedup vs baseline: 1.1705x; 1.1587x over previous
"""Trainium2 Bass kernel for per-pixel kernel application (KPN-style ApplyKernel).

y[c,h,w] = sum_{ii,jj} xpad[c, h+ii, w+jj] * k[ii*11+jj, h, w]

Strategy (8 NeuronCores, data-parallel over H strips of 90 rows):
  - Partition p owns a 10-column block of W (128 partitions x 10 = 1280), with
    the +-5 column halo stored in the free dim, so both row and column shifts
    of a tap are plain access-pattern offsets (DVE lanes are partition-locked,
    so shifts must live in the free dim). All 128 lanes are used.
  - Host: pad x and build bf16 slabs [128, 3ch x 100rows x 20cols] in two
    column-alignment variants so every tap's VectorE read stays 4-byte
    aligned, keeping tensor_tensor in its 2x bf16 mode. k is re-laid-out
    host-side to [128, 121, 900] (partition-block-major, even-column taps
    first) so DMA descriptors are large contiguous chunks.
  - Device, per group of taps: one SWDGE DMA of the k group (f32->bf16 cast
    in flight); per tap one VectorE tensor_tensor multiply (bf16 2x mode) and
    6 TensorE identity-matmuls (K=M=128) accumulating into 6 PSUM banks
    (3 channels x 512/388-col chunks).
  - Epilogue: ScalarE+VectorE evacuate PSUM -> SBUF per channel, overlapped
    per-channel DMAs out, host-side reshape of y.
"""

import sys

if "/opt/trn_rl_repo" not in sys.path:
    sys.path.insert(0, "/opt/trn_rl_repo")

import numpy as np
import ml_dtypes

import concourse.mybir as mybir
from concourse import bacc
from concourse.tile import TileContext
from concourse.bass_utils import run_bass_kernel_spmd

KS = 11
HALF = 5
H, W, C = 720, 1280, 3
NCORES = 8
HS = H // NCORES            # 90 rows per core
NP = 128                    # partitions (one 10-col block each)
CPP = W // NP               # 10 output cols per partition
ROWS_ST = HS + 2 * HALF     # 100 rows stored per partition
COLS_ST = CPP + 2 * HALF    # 20 cols stored per partition
SLABF = C * ROWS_ST * COLS_ST   # 6000 bf16 per partition per variant
NTAPS = KS * KS             # 121
FD = HS * CPP               # 900 elements per channel per tap
PFD = C * FD                # 2700 product elements per tap
N0, N1 = 512, FD - 512      # matmul chunk widths per channel (512, 388)

# tap order: even-jj taps first (only need slab variant 0), then odd-jj
TAP_PERM = ([t for t in range(NTAPS) if (t % KS) % 2 == 0]
            + [t for t in range(NTAPS) if (t % KS) % 2 == 1])
# k DMA group sizes over the permuted order (66 even + 55 odd taps);
# ramped start so the pipeline fills before the big groups are needed
GROUPS = [2, 2, 2, 2, 4, 4, 6] + [8] * 5 + [4] + [8] * 6 + [7]
assert sum(GROUPS) == NTAPS and sum(GROUPS[:13]) == 66

BF16 = ml_dtypes.bfloat16

_CACHE = {}


def _build_nc(taps=NTAPS):
    nc = bacc.Bacc("TRN2", target_bir_lowering=False, debug=False)
    k_d = nc.dram_tensor("k", [NP, NTAPS, FD], mybir.dt.float32, kind="ExternalInput")
    xs_d = nc.dram_tensor("xs", [2, NP, SLABF], mybir.dt.bfloat16, kind="ExternalInput")
    id_d = nc.dram_tensor("ident", [NP, NP], mybir.dt.bfloat16, kind="ExternalInput")
    y_d = nc.dram_tensor("y", [NP, PFD], mybir.dt.float32, kind="ExternalOutput")

    with TileContext(nc) as tc:
        with tc.tile_pool(name="const", bufs=1) as const_pool, \
             tc.tile_pool(name="kbf", bufs=4) as kb_pool, \
             tc.tile_pool(name="prod", bufs=6) as prod_pool, \
             tc.tile_pool(name="out", bufs=1) as out_pool, \
             tc.tile_pool(name="psum", bufs=1, space="PSUM") as psum_pool:

            slab0 = const_pool.tile([NP, SLABF], mybir.dt.bfloat16)
            slab1 = const_pool.tile([NP, SLABF], mybir.dt.bfloat16)
            ident = const_pool.tile([NP, NP], mybir.dt.bfloat16)
            nc.sync.dma_start(ident[:], id_d.ap())
            nc.sync.dma_start(slab0[:], xs_d.ap()[0])
            nc.sync.dma_start(slab1[:], xs_d.ap()[1])
            slab_views = [
                s[:].rearrange("p (c r w) -> p c r w", c=C, r=ROWS_ST, w=COLS_ST)
                for s in (slab0, slab1)]

            accs = []
            for c in range(C):
                a0 = psum_pool.tile([NP, N0], mybir.dt.float32, name=f"acc{c}0")
                a1 = psum_pool.tile([NP, N1], mybir.dt.float32, name=f"acc{c}1")
                accs.append((a0, a1))

            gi0 = 0
            groups = []
            for ng in GROUPS:
                if gi0 >= taps:
                    break
                groups.append((gi0, min(ng, taps - gi0)))
                gi0 += ng
            for gi0, ng in groups:
                kb = kb_pool.tile([NP, ng * FD], mybir.dt.bfloat16, name="kb")
                nc.gpsimd.dma_start(
                    kb[:].rearrange("p (t f) -> p t f", t=ng),
                    k_d.ap()[:, gi0:gi0 + ng, :])

                for dt_ in range(ng):
                    gi = gi0 + dt_
                    t = TAP_PERM[gi]
                    ii, jj = divmod(t, KS)
                    v = jj & 1
                    jj2 = jj - v
                    xs_op = slab_views[v][:, :, ii:ii + HS, jj2:jj2 + CPP]
                    k_op = (kb[:, dt_ * FD:(dt_ + 1) * FD]
                            .rearrange("p (r w) -> p r w", r=HS)
                            .unsqueeze(1).broadcast_to([NP, C, HS, CPP]))
                    prod = prod_pool.tile([NP, PFD], mybir.dt.bfloat16, name="prod")
                    prod_view = prod[:].rearrange(
                        "p (c r w) -> p c r w", c=C, r=HS, w=CPP)
                    nc.vector.tensor_tensor(prod_view, xs_op, k_op,
                                            mybir.AluOpType.mult)
                    first = (gi == 0)
                    last = (gi == taps - 1)
                    for c in range(C):
                        nc.tensor.matmul(accs[c][0][:], ident[:],
                                         prod[:, c * FD:c * FD + N0],
                                         start=first, stop=last)
                        nc.tensor.matmul(accs[c][1][:], ident[:],
                                         prod[:, c * FD + N0:(c + 1) * FD],
                                         start=first, stop=last)

            yst = out_pool.tile([NP, PFD], mybir.dt.float32)
            for c in range(C):
                nc.scalar.copy(yst[:, c * FD:c * FD + N0], accs[c][0][:])
                nc.vector.tensor_copy(yst[:, c * FD + N0:(c + 1) * FD],
                                      accs[c][1][:])
                nc.sync.dma_start(y_d.ap()[:, c * FD:(c + 1) * FD],
                                  yst[:, c * FD:(c + 1) * FD])

    nc.compile()
    return nc


def get_nc(taps=NTAPS):
    if taps not in _CACHE:
        _CACHE[taps] = _build_nc(taps)
    return _CACHE[taps]


def _prep_inputs(x, k, padding, padding_value):
    """Host-side prep: pad x, build bf16 slabs + per-core shards."""
    x = np.asarray(x, dtype=np.float32)
    k = np.asarray(k, dtype=np.float32)
    pad = bool(int(np.asarray(padding)))
    pv = float(np.asarray(padding_value))

    if pad:
        assert x.shape == (1, C, H, W), x.shape
        xp = np.full((C, H + 2 * HALF, W + 2 * HALF + 1), 0.0, dtype=np.float32)
        xp[:, :, :W + 2 * HALF] = pv
        xp[:, HALF:HALF + H, HALF:HALF + W] = x[0]
    else:
        assert x.shape == (1, C, H + 2 * HALF, W + 2 * HALF), x.shape
        xp = np.zeros((C, H + 2 * HALF, W + 2 * HALF + 1), dtype=np.float32)
        xp[:, :, :W + 2 * HALF] = x[0]

    assert k.shape == (1, NTAPS, H, W), k.shape
    # partition-block-major, tap-permuted k: [core, p, t, (r w)]
    kt_all = np.ascontiguousarray(
        k[0][TAP_PERM].reshape(NTAPS, NCORES, HS, NP, CPP)
        .transpose(1, 3, 0, 2, 4)).reshape(NCORES, NP, NTAPS, FD)

    cols_idx = CPP * np.arange(NP)[:, None] + np.arange(COLS_ST)[None, :]
    ident = np.eye(NP, dtype=BF16)
    in_maps = []
    for ci in range(NCORES):
        rows = slice(HS * ci, HS * ci + ROWS_ST)
        xs = np.empty((2, NP, SLABF), dtype=BF16)
        for v in (0, 1):
            sv = xp[:, rows, v:v + W + 2 * HALF]           # [C, 100, 1290]
            win = sv[:, :, cols_idx]                       # [C, 100, 128, 20]
            xs[v] = win.transpose(2, 0, 1, 3).reshape(NP, SLABF).astype(BF16)
        in_maps.append({"k": kt_all[ci], "xs": xs, "ident": ident})
    return in_maps


def _assemble_y(results):
    """results[ci]["y"] is [128, 2700]; reassemble to [1, C, H, W]."""
    y = np.empty((C, H, W), dtype=np.float32)
    for ci in range(NCORES):
        blk = results[ci]["y"].reshape(NP, C, HS, CPP)     # [p, c, r, w]
        y[:, HS * ci:HS * (ci + 1), :] = (
            blk.transpose(1, 2, 0, 3).reshape(C, HS, W))
    return y[None]


def kernel(x, k, padding, padding_value):
    in_maps = _prep_inputs(x, k, padding, padding_value)
    nc = get_nc()
    res = run_bass_kernel_spmd(nc, in_maps, core_ids=list(range(NCORES)))
    return _assemble_y(res.results).astype(np.float32)
